# revision 10
# baseline (speedup 1.0000x reference)
"""Bidirectional LSTM chunk-boundary predictor on 8 Trainium2 NeuronCores.

Strategy (sequence-parallel with halo warm-up), v2:
  - T=65536 tokens split into 8 per-core slices of 8192; each core splits its
    slice into S=256 chunks of L=32 tokens processed in parallel across the
    PSUM/SBUF free dimension. LSTM state forgets exponentially, so each chunk
    warms up on W extra tokens before its region. Fewer, fatter steps (48 vs
    80) cut the serial-chain wall: wall ~= steps x chain_latency.
  - Embedding lookup + input projection + bias are constant-folded on the host
    into PG = w_ih @ E.T + b  [512 x 256] per direction, so on-device the
    per-step input contribution is a one-hot matmul (2 vocab halves).
    Out-of-range halo positions get all-zero one-hot columns, which keeps
    (h,c) exactly zero through warm-up (g-gate preact 0 -> c,h stay 0).
  - All four gates use a single Tanh activation per step via
    sigma(x) = (tanh(x/2)+1)/2: the ACT instruction applies scale=0.5 and the
    g-gate rows of PG/w_hh are pre-scaled by 2. States are kept scaled
    (c' = 2c, h' = 2h) so the cell update is 4 fused scalar_tensor_tensor ops
    with no extra fixup; w_hh and w_out are pre-divided by 2 to compensate.
  - acts/A/tch/h are bf16 (DVE 2x mode where both operands are bf16);
    c stays f32.
  - Scores: [L, S] PSUM tile; per step a single M=1 matmul per direction
    with stationary w_out (1-column LDWEIGHTS) streams h as rhs.
  - Emission order per step keeps the two directions' chains independent on
    every engine queue: gates-ACT d0, d1 -> cell-update DVE d0, d1 ->
    tanh-c ACT d0, d1 -> h DVE d0, d1.
"""

import sys

sys.path.insert(0, "/opt/trn_rl_repo")

import numpy as np

H = 128
VOCAB = 256
N_CORES = 8

S = 256   # chunks per core (free-dim parallelism)
L = 32    # tokens per chunk
W = 16    # halo warm-up tokens


def _build_nc(S, L, W):
    import concourse.bass as bass
    import concourse.bacc as bacc
    import concourse.mybir as mybir
    import concourse.tile as tile

    f32 = mybir.dt.float32
    bf16 = mybir.dt.bfloat16
    n2 = (L + 2 * W) * S       # step-major one-hot columns per vocab half
    steps = L + W

    nc = bacc.Bacc(None, target_bir_lowering=False)
    oh_d = nc.declare_dram_parameter("oh", [128, 2 * n2], bf16, isOutput=False)
    pg_d = nc.declare_dram_parameter("pg", [128, 16 * 128], bf16, isOutput=False)
    whh_d = nc.declare_dram_parameter("whh", [128, 8 * 128], bf16, isOutput=False)
    wscb_d = nc.declare_dram_parameter("wscb", [128, 2], bf16, isOutput=False)
    wsc32_d = nc.declare_dram_parameter("wsc32", [128, 1], f32, isOutput=False)
    out_d = nc.declare_dram_parameter("out", [128, 2 * L], f32, isOutput=True)

    TANH = mybir.ActivationFunctionType.Tanh
    SIGM = mybir.ActivationFunctionType.Sigmoid
    ADD = mybir.AluOpType.add
    MULT = mybir.AluOpType.mult

    with tile.TileContext(nc) as tc:
        with (
            tc.tile_pool(name="singles", bufs=1) as singles,
            tc.tile_pool(name="acts", bufs=2) as apool,
            tc.tile_pool(name="hpool", bufs=2) as hpool,
            tc.tile_pool(name="tmp", bufs=2) as tpool,
            tc.tile_pool(name="gates", bufs=1, space="PSUM") as gpool,
            tc.tile_pool(name="scps", bufs=1, space="PSUM") as scpool,
        ):
            ohtw = n2 // 4                      # columns per one-hot tile
            oht = []
            for k in range(8):
                o_k = singles.tile([128, ohtw], bf16, tag=f"oh{k}",
                                   name=f"oh{k}")
                oht.append(o_k)
            pg = singles.tile([128, 16 * 128], bf16)
            whh = singles.tile([128, 8 * 128], bf16)
            wscb = singles.tile([128, 2], bf16)
            wsc32 = singles.tile([128, 1], f32)
            zrow = singles.tile([1, S], f32)
            scr = singles.tile([1, 1], f32)           # ACT prime scratch
            scr2 = singles.tile([1, 1], f32)          # ACT prime scratch 2
            out_sb = singles.tile([128, 2 * L], f32)
            # per-direction persistent state (independent dep chains)
            cst = []
            tch = []
            for d in range(2):
                c_d = singles.tile([128, S], f32, tag=f"c{d}", name=f"c{d}")
                t_d = singles.tile([128, S], bf16, tag=f"tch{d}", name=f"tch{d}")
                cst.append(c_d)
                tch.append(t_d)

            # input DMAs; both ends of each vocab half first, since the
            # forward direction consumes columns from the front and the
            # reverse direction from the back
            oh_slices = []
            for k in (0, 4, 3, 7, 1, 5, 2, 6):
                half, q = k // 4, k % 4
                a = half * n2 + q * ohtw
                nc.sync.dma_start(oht[k][:], oh_d[:, a:a + ohtw])
                oh_slices.append(oht[k][:, 0:1])
            nc.sync.dma_start(pg[:], pg_d[:])
            nc.sync.dma_start(whh[:], whh_d[:])
            nc.sync.dma_start(wscb[:], wscb_d[:])
            nc.sync.dma_start(wsc32[:], wsc32_d[:])

            for d in range(2):
                nc.vector.memset(cst[d][:], 0.0)
            nc.vector.memset(zrow[:], 0.0)

            bias0 = nc.const_aps.scalar_like(0.0, oht[0][:, 0:1])

            # scores psum ([128, 2, L]: chunk-row, chunk-half, position);
            # prime matmuls write into it before the zero-seed wipes the
            # bank, so no separate prime bank is needed.
            scores = scpool.tile([128, 2, L], f32)

            # prime PE on every DMA'd tensor (walrus allows 1 sync-wait/inst,
            # so each engine must observe each producer semaphore separately)
            for ap in oh_slices + [pg[:, 0:1], whh[:, 0:1], wscb[:, 0:1],
                                   wsc32[:, 0:1]]:
                nc.tensor.matmul(scores[0:1, 0, 0:1], ap[0:1, 0:1],
                                 ap[0:1, 0:1],
                                 start=True, stop=True, skip_group_check=True)
            # prime ACT: first on the const-bias AP alone (input==bias, one
            # producer), then on wsc32 (b_out bias for the final sigmoid)
            nc.scalar.activation(scr[:], bias0[0:1, :], TANH, bias=bias0[0:1, :])
            nc.scalar.activation(scr2[:], wsc32[0:1, 0:1], TANH, bias=bias0[0:1, :])

            # zero-seed the scores psum so both directions can accumulate
            # columns in any order with start=False afterwards
            nc.tensor.matmul(scores[:], zrow[0:1, 0:128], zrow[0:1, 0:2 * L],
                             start=True, stop=True, skip_group_check=True)

            hs = []
            for d in range(2):
                h_d = hpool.tile([128, S], bf16, tag=f"h{d}", name=f"h{d}")
                hs.append(h_d)
            nc.vector.memset(hs[0][:], 0.0)
            nc.vector.memset(hs[1][:], 0.0)

            # Per step and direction one psum tile [128, 4, S] (2 banks:
            # gates i,f in bank A, g,o in bank B). The input-projection
            # (one-hot) matmuls for step t+1 are emitted between this step's
            # recurrent matmuls and the chain tail so the PE has ready work
            # while h is being produced; with bufs=1 they wait (WAR) on this
            # step's gates-ACT only, which finishes early in the chain.
            def emit_xg(t):
                tiles = []
                for d in range(2):
                    g_ps = gpool.tile([128, 4, S], f32, tag=f"g{d}",
                                      name=f"g{d}_{t}")
                    off = t if d == 0 else (L + 2 * W - 1 - t)
                    q0 = off * S
                    for g in range(4):
                        for half in range(2):
                            lhs = pg[:, ((d * 4 + g) * 2 + half) * 128:
                                        ((d * 4 + g) * 2 + half + 1) * 128]
                            rhs = oht[half * 4 + q0 // ohtw][
                                :, q0 % ohtw:q0 % ohtw + S]
                            # start=True on the first write of each psum bank
                            nc.tensor.matmul(g_ps[:, g, :], lhs,
                                             rhs, start=(half == 0 and g % 2 == 0),
                                             stop=False,
                                             skip_group_check=True)
                    tiles.append(g_ps)
                return tiles

            cur = emit_xg(0)
            for t in range(steps):
                for d in range(2):
                    for g in range(4):
                        nc.tensor.matmul(
                            cur[d][:, g, :],
                            whh[:, (d * 4 + g) * 128:(d * 4 + g + 1) * 128],
                            hs[d][:], start=False, stop=True,
                            skip_group_check=True)
                nxt = emit_xg(t + 1) if t + 1 < steps else None
                acts = []
                for d in range(2):
                    a_d = apool.tile([128, 4, S], bf16, tag=f"acts{d}",
                                     name=f"acts{d}_{t}")
                    nc.scalar.activation(a_d[:], cur[d][:], TANH,
                                         bias=bias0, scale=0.5)
                    acts.append(a_d)
                ABs = []
                for d in range(2):
                    yi = acts[d][:, 0, :]
                    yf = acts[d][:, 1, :]
                    yg = acts[d][:, 2, :]
                    c = cst[d]
                    # c' = (yf+1)*c'*0.5 + (yi+1)*yg
                    A = tpool.tile([128, S], bf16, tag=f"A{d}", name=f"A{d}_{t}")
                    Bt = tpool.tile([128, S], f32, tag=f"B{d}", name=f"B{d}_{t}")
                    nc.vector.scalar_tensor_tensor(A[:], yi, 1.0, yg,
                                                   op0=ADD, op1=MULT)
                    nc.vector.scalar_tensor_tensor(Bt[:], yf, 1.0, c[:],
                                                   op0=ADD, op1=MULT)
                    nc.vector.scalar_tensor_tensor(c[:], Bt[:], 0.5, A[:],
                                                   op0=MULT, op1=ADD)
                    ABs.append((A, Bt))
                for d in range(2):
                    # tanh(c) = tanh(0.5 * c')
                    nc.scalar.activation(tch[d][:], cst[d][:], TANH,
                                         bias=bias0, scale=0.5)
                h_new = [None, None]
                for d in range(2):
                    yo = acts[d][:, 3, :]
                    h_d = hpool.tile([128, S], bf16, tag=f"h{d}",
                                     name=f"h{d}_{t}")
                    # h' = (yo+1)*tanh(c)
                    nc.vector.scalar_tensor_tensor(h_d[:], yo, 1.0,
                                                   tch[d][:],
                                                   op0=ADD, op1=MULT)
                    h_new[d] = h_d
                for d in range(2):
                    hs[d] = h_new[d]
                    # scores: s[:, half, p] += h_half.T @ w_out_dir
                    if t >= W:
                        p = (t - W) if d == 0 else (L + W - 1 - t)
                        for half in range(2):
                            nc.tensor.matmul(
                                scores[:, half, p:p + 1],
                                hs[d][:, half * 128:(half + 1) * 128],
                                wscb[:, d:d + 1], start=False,
                                stop=True, skip_group_check=True)
                cur = nxt

            # --- epilogue: sigmoid(scores + b_out) and store ---
            nc.scalar.activation(out_sb[:], scores[:], SIGM,
                                 bias=wsc32[:, 0:1])
            nc.sync.dma_start(out_d[:], out_sb[:])

    nc.compile()
    return nc


def _host_prep(inputs, S, L, W):
    """Build per-core in_maps."""
    import ml_dtypes

    bf16 = ml_dtypes.bfloat16

    tokens = np.asarray(inputs["tokens"]).astype(np.int64)
    emb = np.asarray(inputs["embedding"], dtype=np.float32)
    T = tokens.shape[0]
    n2 = (L + 2 * W) * S

    pg_blob = np.zeros((128, 16 * 128), np.float32)
    whh_blob = np.zeros((128, 8 * 128), np.float32)
    for d, sfx in enumerate(("f", "r")):
        w_ih = np.asarray(inputs[f"w_ih_{sfx}"], dtype=np.float32)
        w_hh = np.asarray(inputs[f"w_hh_{sfx}"], dtype=np.float32)
        b = (np.asarray(inputs[f"b_ih_{sfx}"], dtype=np.float32)
             + np.asarray(inputs[f"b_hh_{sfx}"], dtype=np.float32))
        PG = w_ih @ emb.T + b[:, None]          # [512, 256]
        PG[2 * H:3 * H] *= 2.0                  # tanh-trick on g-gate
        whh = w_hh * 0.5                        # h' = 2h compensation
        whh[2 * H:3 * H] *= 2.0                 # tanh-trick on g-gate
        for g in range(4):
            for half in range(2):
                tilev = PG[g * 128:(g + 1) * 128, half * 128:(half + 1) * 128].T
                pg_blob[:, ((d * 4 + g) * 2 + half) * 128:
                           ((d * 4 + g) * 2 + half + 1) * 128] = tilev
            whh_blob[:, (d * 4 + g) * 128:(d * 4 + g + 1) * 128] = \
                whh[g * 128:(g + 1) * 128, :].T

    w_out = np.asarray(inputs["w_out"], dtype=np.float32).reshape(-1)
    b_out = float(np.asarray(inputs["b_out"]).reshape(-1)[0])
    wscb = np.stack([w_out[:H] * 0.5, w_out[H:] * 0.5], axis=1)  # [128, 2]
    wsc32 = np.full((128, 1), b_out, np.float32)

    pg16 = pg_blob.astype(bf16)
    whhb = whh_blob.astype(bf16)
    wscbb = wscb.astype(bf16)

    in_maps = []
    idxg, sg = np.meshgrid(np.arange(L + 2 * W), np.arange(S), indexing="ij")
    colg = (idxg * S + sg).reshape(-1)          # step-major column index
    for core in range(N_CORES):
        base = core * S * L
        pos = (base + sg * L + idxg - W).reshape(-1)
        valid = (pos >= 0) & (pos < T)
        cols = colg[valid]
        toks = tokens[pos[valid]]
        ohc = np.zeros((2, 128, n2), np.float32)
        lo = toks < 128
        ohc[0, toks[lo], cols[lo]] = 1.0
        ohc[1, toks[~lo] - 128, cols[~lo]] = 1.0
        oh = np.concatenate([ohc[0], ohc[1]], axis=1).astype(bf16)  # [128, 2n2]
        in_maps.append({
            "oh": oh,
            "pg": pg16,
            "whh": whhb,
            "wscb": wscbb,
            "wsc32": wsc32,
        })
    return in_maps


_CACHE = {}


def kernel(**inputs):
    from concourse.bass_utils import run_bass_kernel_spmd

    key = (S, L, W)
    if key not in _CACHE:
        _CACHE[key] = _build_nc(S, L, W)
    nc = _CACHE[key]
    in_maps = _host_prep(inputs, S, L, W)
    res = run_bass_kernel_spmd(nc, in_maps, list(range(N_CORES)))
    # out tile is [128, 2, L]: (chunk-row, chunk-half, position);
    # chunk s = half*128 + row, token = core_base + s*L + p
    out = np.concatenate(
        [np.asarray(res.results[c]["out"], dtype=np.float32)
         .reshape(128, 2, L).transpose(1, 0, 2).reshape(-1)
         for c in range(N_CORES)])
    return out


def run_traced(inputs):
    """Run once with NTFF tracing for HW timing / perfetto (dev only)."""
    from concourse.bass_utils import run_bass_kernel_spmd

    key = (S, L, W)
    if key not in _CACHE:
        _CACHE[key] = _build_nc(S, L, W)
    nc = _CACHE[key]
    in_maps = _host_prep(inputs, S, L, W)
    return run_bass_kernel_spmd(nc, in_maps, list(range(N_CORES)), trace=True)


# revision 13
# speedup vs baseline: 1.3962x; 1.3962x over previous
"""Bidirectional LSTM chunk-boundary predictor on 8 Trainium2 NeuronCores.

Strategy (sequence-parallel with halo warm-up), v2:
  - T=65536 tokens split into 8 per-core slices of 8192; each core splits its
    slice into S=256 chunks of L=32 tokens processed in parallel across the
    PSUM/SBUF free dimension. LSTM state forgets exponentially, so each chunk
    warms up on W extra tokens before its region. Fewer, fatter steps (48 vs
    80) cut the serial-chain wall: wall ~= steps x chain_latency.
  - Embedding lookup + input projection + bias are constant-folded on the host
    into PG = w_ih @ E.T + b  [512 x 256] per direction, so on-device the
    per-step input contribution is a one-hot matmul (2 vocab halves).
    Out-of-range halo positions get all-zero one-hot columns, which keeps
    (h,c) exactly zero through warm-up (g-gate preact 0 -> c,h stay 0).
  - All four gates use a single Tanh activation per step via
    sigma(x) = (tanh(x/2)+1)/2: the ACT instruction applies scale=0.5 and the
    g-gate rows of PG/w_hh are pre-scaled by 2. States are kept scaled
    (c' = 2c, h' = 2h) so the cell update is 4 fused scalar_tensor_tensor ops
    with no extra fixup; w_hh and w_out are pre-divided by 2 to compensate.
  - acts/A/tch/h are bf16 (DVE 2x mode where both operands are bf16);
    c stays f32.
  - Scores: [L, S] PSUM tile; per step a single M=1 matmul per direction
    with stationary w_out (1-column LDWEIGHTS) streams h as rhs.
  - Emission order per step keeps the two directions' chains independent on
    every engine queue: gates-ACT d0, d1 -> cell-update DVE d0, d1 ->
    tanh-c ACT d0, d1 -> h DVE d0, d1.
"""

import sys

sys.path.insert(0, "/opt/trn_rl_repo")

import numpy as np

H = 128
VOCAB = 256
N_CORES = 8

S = 256   # chunks per core (free-dim parallelism)
L = 32    # tokens per chunk
W = 8     # halo warm-up tokens


def _build_nc(S, L, W):
    import concourse.bass as bass
    import concourse.bacc as bacc
    import concourse.mybir as mybir
    import concourse.tile as tile

    f32 = mybir.dt.float32
    bf16 = mybir.dt.bfloat16
    n2 = (L + 2 * W) * S       # step-major one-hot columns per vocab half
    steps = L + W

    nc = bacc.Bacc(None, target_bir_lowering=False)
    oh_d = nc.declare_dram_parameter("oh", [128, 2 * n2], bf16, isOutput=False)
    pg_d = nc.declare_dram_parameter("pg", [128, 16 * 128], bf16, isOutput=False)
    whh_d = nc.declare_dram_parameter("whh", [128, 8 * 128], bf16, isOutput=False)
    wscb_d = nc.declare_dram_parameter("wscb", [128, 2], bf16, isOutput=False)
    wsc32_d = nc.declare_dram_parameter("wsc32", [128, 1], f32, isOutput=False)
    out_d = nc.declare_dram_parameter("out", [128, 2 * L], f32, isOutput=True)

    TANH = mybir.ActivationFunctionType.Tanh
    SIGM = mybir.ActivationFunctionType.Sigmoid
    ADD = mybir.AluOpType.add
    MULT = mybir.AluOpType.mult

    with tile.TileContext(nc) as tc:
        with (
            tc.tile_pool(name="singles", bufs=1) as singles,
            tc.tile_pool(name="acts", bufs=2) as apool,
            tc.tile_pool(name="hpool", bufs=2) as hpool,
            tc.tile_pool(name="tmp", bufs=2) as tpool,
            tc.tile_pool(name="gates", bufs=1, space="PSUM") as gpool,
            tc.tile_pool(name="scps", bufs=1, space="PSUM") as scpool,
        ):
            ohtw = n2 // 4                      # columns per one-hot tile
            oht = []
            for k in range(8):
                o_k = singles.tile([128, ohtw], bf16, tag=f"oh{k}",
                                   name=f"oh{k}")
                oht.append(o_k)
            pg = singles.tile([128, 16 * 128], bf16)
            whh = singles.tile([128, 8 * 128], bf16)
            wscb = singles.tile([128, 2], bf16)
            wsc32 = singles.tile([128, 1], f32)
            zrow = singles.tile([1, S], f32)
            scr = singles.tile([1, 1], f32)           # ACT prime scratch
            scr2 = singles.tile([1, 1], f32)          # ACT prime scratch 2
            out_sb = singles.tile([128, 2 * L], f32)
            # per-direction persistent state (independent dep chains)
            cst = []
            tch = []
            for d in range(2):
                c_d = singles.tile([128, S], f32, tag=f"c{d}", name=f"c{d}")
                t_d = singles.tile([128, S], bf16, tag=f"tch{d}", name=f"tch{d}")
                cst.append(c_d)
                tch.append(t_d)

            # input DMAs; both ends of each vocab half first, since the
            # forward direction consumes columns from the front and the
            # reverse direction from the back
            oh_slices = []
            for k in (0, 4, 3, 7, 1, 5, 2, 6):
                half, q = k // 4, k % 4
                a = half * n2 + q * ohtw
                nc.sync.dma_start(oht[k][:], oh_d[:, a:a + ohtw])
                oh_slices.append(oht[k][:, 0:1])
            nc.sync.dma_start(pg[:], pg_d[:])
            nc.sync.dma_start(whh[:], whh_d[:])
            nc.sync.dma_start(wscb[:], wscb_d[:])
            nc.sync.dma_start(wsc32[:], wsc32_d[:])

            for d in range(2):
                nc.vector.memset(cst[d][:], 0.0)
            nc.vector.memset(zrow[:], 0.0)

            bias0 = nc.const_aps.scalar_like(0.0, oht[0][:, 0:1])

            # scores psum ([128, 2, L]: chunk-row, chunk-half, position);
            # prime matmuls write into it before the zero-seed wipes the
            # bank, so no separate prime bank is needed.
            scores = scpool.tile([128, 2, L], f32)

            # prime PE on every DMA'd tensor (walrus allows 1 sync-wait/inst,
            # so each engine must observe each producer semaphore separately)
            for ap in oh_slices + [pg[:, 0:1], whh[:, 0:1], wscb[:, 0:1],
                                   wsc32[:, 0:1]]:
                nc.tensor.matmul(scores[0:1, 0, 0:1], ap[0:1, 0:1],
                                 ap[0:1, 0:1],
                                 start=True, stop=True, skip_group_check=True)
            # prime ACT: first on the const-bias AP alone (input==bias, one
            # producer), then on wsc32 (b_out bias for the final sigmoid)
            nc.scalar.activation(scr[:], bias0[0:1, :], TANH, bias=bias0[0:1, :])
            nc.scalar.activation(scr2[:], wsc32[0:1, 0:1], TANH, bias=bias0[0:1, :])

            # zero-seed the scores psum so both directions can accumulate
            # columns in any order with start=False afterwards
            nc.tensor.matmul(scores[:], zrow[0:1, 0:128], zrow[0:1, 0:2 * L],
                             start=True, stop=True, skip_group_check=True)

            hs = []
            for d in range(2):
                h_d = hpool.tile([128, S], bf16, tag=f"h{d}", name=f"h{d}")
                hs.append(h_d)
            nc.vector.memset(hs[0][:], 0.0)
            nc.vector.memset(hs[1][:], 0.0)

            # Per step and direction one psum tile [128, 4, S] (2 banks:
            # gates i,f in bank A, g,o in bank B). dir 0 gets bufs=2 so its
            # step-t+1 one-hot matmuls never wait on the step-t gates-ACT;
            # dir 1 (bufs=1, PSUM is full) emits its xg at the end of its
            # block where the WAR wait on its own gates-ACT is already met.
            # The two directions are emitted SERIALLY (f-block then r-block)
            # so they settle a half-period apart: each direction's
            # DVE/ACT work fills the other's h->gates reload latency.
            gbufs = [2, 1]

            def emit_xg(d, t):
                g_ps = gpool.tile([128, 4, S], f32, tag=f"g{d}",
                                  name=f"g{d}_{t}", bufs=gbufs[d])
                off = t if d == 0 else (L + 2 * W - 1 - t)
                q0 = off * S
                for g in range(4):
                    for half in range(2):
                        lhs = pg[:, ((d * 4 + g) * 2 + half) * 128:
                                    ((d * 4 + g) * 2 + half + 1) * 128]
                        rhs = oht[half * 4 + q0 // ohtw][
                            :, q0 % ohtw:q0 % ohtw + S]
                        # start=True on the first write of each psum bank
                        nc.tensor.matmul(g_ps[:, g, :], lhs, rhs,
                                         start=(half == 0 and g % 2 == 0),
                                         stop=False,
                                         skip_group_check=True)
                return g_ps

            cur = [emit_xg(0, 0), emit_xg(1, 0)]
            for t in range(steps):
                for d in range(2):
                    # --- dir-d block: recurrent MM -> gates ACT -> cell ---
                    for g in range(4):
                        nc.tensor.matmul(
                            cur[d][:, g, :],
                            whh[:, (d * 4 + g) * 128:(d * 4 + g + 1) * 128],
                            hs[d][:], start=False, stop=True,
                            skip_group_check=True)
                    if d == 0 and t + 1 < steps:
                        nxt0 = emit_xg(0, t + 1)
                    acts = apool.tile([128, 4, S], bf16, tag=f"acts{d}",
                                      name=f"acts{d}_{t}")
                    nc.scalar.activation(acts[:], cur[d][:], TANH,
                                         bias=bias0, scale=0.5)
                    yi = acts[:, 0, :]
                    yf = acts[:, 1, :]
                    yg = acts[:, 2, :]
                    yo = acts[:, 3, :]
                    c = cst[d]
                    # c' = (yf+1)*c'*0.5 + (yi+1)*yg
                    A = tpool.tile([128, S], bf16, tag=f"A{d}", name=f"A{d}_{t}")
                    Bt = tpool.tile([128, S], f32, tag=f"B{d}", name=f"B{d}_{t}")
                    nc.vector.scalar_tensor_tensor(A[:], yi, 1.0, yg,
                                                   op0=ADD, op1=MULT)
                    nc.vector.scalar_tensor_tensor(Bt[:], yf, 1.0, c[:],
                                                   op0=ADD, op1=MULT)
                    nc.vector.scalar_tensor_tensor(c[:], Bt[:], 0.5, A[:],
                                                   op0=MULT, op1=ADD)
                    # tanh(c) = tanh(0.5 * c')
                    nc.scalar.activation(tch[d][:], c[:], TANH,
                                         bias=bias0, scale=0.5)
                    h_d = hpool.tile([128, S], bf16, tag=f"h{d}",
                                     name=f"h{d}_{t}")
                    # h' = (yo+1)*tanh(c)
                    nc.vector.scalar_tensor_tensor(h_d[:], yo, 1.0,
                                                   tch[d][:],
                                                   op0=ADD, op1=MULT)
                    hs[d] = h_d
                    # scores: s[:, half, p] += h_half.T @ w_out_dir
                    if t >= W:
                        p = (t - W) if d == 0 else (L + W - 1 - t)
                        for half in range(2):
                            nc.tensor.matmul(
                                scores[:, half, p:p + 1],
                                hs[d][:, half * 128:(half + 1) * 128],
                                wscb[:, d:d + 1], start=False,
                                stop=True, skip_group_check=True)
                    if d == 1 and t + 1 < steps:
                        cur = [nxt0, emit_xg(1, t + 1)]

            # --- epilogue: sigmoid(scores + b_out) and store ---
            nc.scalar.activation(out_sb[:], scores[:], SIGM,
                                 bias=wsc32[:, 0:1])
            nc.sync.dma_start(out_d[:], out_sb[:])

    nc.compile()
    return nc


def _host_prep(inputs, S, L, W):
    """Build per-core in_maps."""
    import ml_dtypes

    bf16 = ml_dtypes.bfloat16

    tokens = np.asarray(inputs["tokens"]).astype(np.int64)
    emb = np.asarray(inputs["embedding"], dtype=np.float32)
    T = tokens.shape[0]
    n2 = (L + 2 * W) * S

    pg_blob = np.zeros((128, 16 * 128), np.float32)
    whh_blob = np.zeros((128, 8 * 128), np.float32)
    for d, sfx in enumerate(("f", "r")):
        w_ih = np.asarray(inputs[f"w_ih_{sfx}"], dtype=np.float32)
        w_hh = np.asarray(inputs[f"w_hh_{sfx}"], dtype=np.float32)
        b = (np.asarray(inputs[f"b_ih_{sfx}"], dtype=np.float32)
             + np.asarray(inputs[f"b_hh_{sfx}"], dtype=np.float32))
        PG = w_ih @ emb.T + b[:, None]          # [512, 256]
        PG[2 * H:3 * H] *= 2.0                  # tanh-trick on g-gate
        whh = w_hh * 0.5                        # h' = 2h compensation
        whh[2 * H:3 * H] *= 2.0                 # tanh-trick on g-gate
        for g in range(4):
            for half in range(2):
                tilev = PG[g * 128:(g + 1) * 128, half * 128:(half + 1) * 128].T
                pg_blob[:, ((d * 4 + g) * 2 + half) * 128:
                           ((d * 4 + g) * 2 + half + 1) * 128] = tilev
            whh_blob[:, (d * 4 + g) * 128:(d * 4 + g + 1) * 128] = \
                whh[g * 128:(g + 1) * 128, :].T

    w_out = np.asarray(inputs["w_out"], dtype=np.float32).reshape(-1)
    b_out = float(np.asarray(inputs["b_out"]).reshape(-1)[0])
    wscb = np.stack([w_out[:H] * 0.5, w_out[H:] * 0.5], axis=1)  # [128, 2]
    wsc32 = np.full((128, 1), b_out, np.float32)

    pg16 = pg_blob.astype(bf16)
    whhb = whh_blob.astype(bf16)
    wscbb = wscb.astype(bf16)

    in_maps = []
    idxg, sg = np.meshgrid(np.arange(L + 2 * W), np.arange(S), indexing="ij")
    colg = (idxg * S + sg).reshape(-1)          # step-major column index
    for core in range(N_CORES):
        base = core * S * L
        pos = (base + sg * L + idxg - W).reshape(-1)
        valid = (pos >= 0) & (pos < T)
        cols = colg[valid]
        toks = tokens[pos[valid]]
        ohc = np.zeros((2, 128, n2), np.float32)
        lo = toks < 128
        ohc[0, toks[lo], cols[lo]] = 1.0
        ohc[1, toks[~lo] - 128, cols[~lo]] = 1.0
        oh = np.concatenate([ohc[0], ohc[1]], axis=1).astype(bf16)  # [128, 2n2]
        in_maps.append({
            "oh": oh,
            "pg": pg16,
            "whh": whhb,
            "wscb": wscbb,
            "wsc32": wsc32,
        })
    return in_maps


_CACHE = {}


def kernel(**inputs):
    from concourse.bass_utils import run_bass_kernel_spmd

    key = (S, L, W)
    if key not in _CACHE:
        _CACHE[key] = _build_nc(S, L, W)
    nc = _CACHE[key]
    in_maps = _host_prep(inputs, S, L, W)
    res = run_bass_kernel_spmd(nc, in_maps, list(range(N_CORES)))
    # out tile is [128, 2, L]: (chunk-row, chunk-half, position);
    # chunk s = half*128 + row, token = core_base + s*L + p
    out = np.concatenate(
        [np.asarray(res.results[c]["out"], dtype=np.float32)
         .reshape(128, 2, L).transpose(1, 0, 2).reshape(-1)
         for c in range(N_CORES)])
    return out


def run_traced(inputs):
    """Run once with NTFF tracing for HW timing / perfetto (dev only)."""
    from concourse.bass_utils import run_bass_kernel_spmd

    key = (S, L, W)
    if key not in _CACHE:
        _CACHE[key] = _build_nc(S, L, W)
    nc = _CACHE[key]
    in_maps = _host_prep(inputs, S, L, W)
    return run_bass_kernel_spmd(nc, in_maps, list(range(N_CORES)), trace=True)


# revision 17
# speedup vs baseline: 1.5228x; 1.0907x over previous
"""Bidirectional LSTM chunk-boundary predictor on 8 Trainium2 NeuronCores.

Strategy (sequence-parallel with halo warm-up), v2:
  - T=65536 tokens split into 8 per-core slices of 8192; each core splits its
    slice into S=256 chunks of L=32 tokens processed in parallel across the
    PSUM/SBUF free dimension. LSTM state forgets exponentially, so each chunk
    warms up on W extra tokens before its region. Fewer, fatter steps (48 vs
    80) cut the serial-chain wall: wall ~= steps x chain_latency.
  - Embedding lookup + input projection + bias are constant-folded on the host
    into PG = w_ih @ E.T + b  [512 x 256] per direction, so on-device the
    per-step input contribution is a one-hot matmul (2 vocab halves).
    Out-of-range halo positions get all-zero one-hot columns, which keeps
    (h,c) exactly zero through warm-up (g-gate preact 0 -> c,h stay 0).
  - All four gates use a single Tanh activation per step via
    sigma(x) = (tanh(x/2)+1)/2: the ACT instruction applies scale=0.5 and the
    g-gate rows of PG/w_hh are pre-scaled by 2. States are kept scaled
    (c' = 2c, h' = 2h) so the cell update is 4 fused scalar_tensor_tensor ops
    with no extra fixup; w_hh and w_out are pre-divided by 2 to compensate.
  - acts/A/tch/h are bf16 (DVE 2x mode where both operands are bf16);
    c stays f32.
  - Scores: [L, S] PSUM tile; per step a single M=1 matmul per direction
    with stationary w_out (1-column LDWEIGHTS) streams h as rhs.
  - Emission order per step keeps the two directions' chains independent on
    every engine queue: gates-ACT d0, d1 -> cell-update DVE d0, d1 ->
    tanh-c ACT d0, d1 -> h DVE d0, d1.
"""

import sys

sys.path.insert(0, "/opt/trn_rl_repo")

import numpy as np

H = 128
VOCAB = 256
N_CORES = 8

S = 256   # chunks per core (free-dim parallelism)
L = 32    # tokens per chunk
W = 6     # halo warm-up tokens


def _build_nc(S, L, W):
    import concourse.bass as bass
    import concourse.bacc as bacc
    import concourse.mybir as mybir
    import concourse.tile as tile

    f32 = mybir.dt.float32
    bf16 = mybir.dt.bfloat16
    n2 = (L + 2 * W) * S       # step-major one-hot columns per vocab half
    steps = L + W

    nc = bacc.Bacc(None, target_bir_lowering=False)
    oh_d = nc.declare_dram_parameter("oh", [128, 2 * n2], bf16, isOutput=False)
    pg_d = nc.declare_dram_parameter("pg", [128, 16 * 128], bf16, isOutput=False)
    whh_d = nc.declare_dram_parameter("whh", [128, 8 * 128], bf16, isOutput=False)
    wscb_d = nc.declare_dram_parameter("wscb", [128, 2], bf16, isOutput=False)
    wsc32_d = nc.declare_dram_parameter("wsc32", [128, 1], f32, isOutput=False)
    out_d = nc.declare_dram_parameter("out", [128, 2 * L], f32, isOutput=True)

    TANH = mybir.ActivationFunctionType.Tanh
    SIGM = mybir.ActivationFunctionType.Sigmoid
    ADD = mybir.AluOpType.add
    MULT = mybir.AluOpType.mult

    with tile.TileContext(nc) as tc:
        with (
            tc.tile_pool(name="singles", bufs=1) as singles,
            tc.tile_pool(name="acts", bufs=2) as apool,
            tc.tile_pool(name="hpool", bufs=2) as hpool,
            tc.tile_pool(name="tmp", bufs=2) as tpool,
            tc.tile_pool(name="gates", bufs=1, space="PSUM") as gpool,
            tc.tile_pool(name="scps", bufs=1, space="PSUM") as scpool,
        ):
            ohtw = n2 // 4                      # columns per one-hot tile
            oht = []
            for k in range(8):
                o_k = singles.tile([128, ohtw], bf16, tag=f"oh{k}",
                                   name=f"oh{k}")
                oht.append(o_k)
            pg = singles.tile([128, 16 * 128], bf16)
            whh = singles.tile([128, 8 * 128], bf16)
            wscb = singles.tile([128, 2], bf16)
            wsc32 = singles.tile([128, 1], f32)
            zrow = singles.tile([1, S], f32)
            scr = singles.tile([1, 1], f32)           # ACT prime scratch
            scr2 = singles.tile([1, 1], f32)          # ACT prime scratch 2
            out_sb = singles.tile([128, 2 * L], f32)
            # per-direction persistent state (independent dep chains)
            cst = []
            tch = []
            for d in range(2):
                c_d = singles.tile([128, S], f32, tag=f"c{d}", name=f"c{d}")
                t_d = singles.tile([128, S], bf16, tag=f"tch{d}", name=f"tch{d}")
                cst.append(c_d)
                tch.append(t_d)

            # input DMAs: small weight tensors first (step 0 needs them),
            # then one-hot tiles in consumption order -- both ends of each
            # vocab half first, since the forward direction consumes columns
            # from the front and the reverse direction from the back
            nc.sync.dma_start(pg[:], pg_d[:])
            nc.sync.dma_start(whh[:], whh_d[:])
            nc.sync.dma_start(wscb[:], wscb_d[:])
            nc.sync.dma_start(wsc32[:], wsc32_d[:])
            for k in (0, 4, 3, 7, 1, 5, 2, 6):
                half, q = k // 4, k % 4
                a = half * n2 + q * ohtw
                nc.sync.dma_start(oht[k][:], oh_d[:, a:a + ohtw])

            for d in range(2):
                nc.vector.memset(cst[d][:], 0.0)
            nc.vector.memset(zrow[:], 0.0)

            bias0 = nc.const_aps.scalar_like(0.0, oht[0][:, 0:1])

            # scores psum ([128, 2, L]: chunk-row, chunk-half, position);
            # prime matmuls write into it before the zero-seed wipes the
            # bank, so no separate prime bank is needed.
            scores = scpool.tile([128, 2, L], f32)

            # prime PE on the weight tensors + the 4 first-needed one-hot
            # tiles (walrus allows 1 sync-wait/inst, so each engine must
            # observe each producer semaphore separately). The primes for
            # the late-arriving oh tiles are deferred into the loop so the
            # steady state starts as soon as the first tiles land.
            for ap in [pg[:, 0:1], whh[:, 0:1], wscb[:, 0:1], wsc32[:, 0:1],
                       oht[0][:, 0:1], oht[4][:, 0:1], oht[3][:, 0:1],
                       oht[7][:, 0:1]]:
                nc.tensor.matmul(scores[0:1, 0, 0:1], ap[0:1, 0:1],
                                 ap[0:1, 0:1],
                                 start=True, stop=True, skip_group_check=True)
            primed = {0, 4, 3, 7}
            # prime ACT: first on the const-bias AP alone (input==bias, one
            # producer), then on wsc32 (b_out bias for the final sigmoid)
            nc.scalar.activation(scr[:], bias0[0:1, :], TANH, bias=bias0[0:1, :])
            nc.scalar.activation(scr2[:], wsc32[0:1, 0:1], TANH, bias=bias0[0:1, :])

            # zero-seed the scores psum so both directions can accumulate
            # columns in any order with start=False afterwards
            nc.tensor.matmul(scores[:], zrow[0:1, 0:128], zrow[0:1, 0:2 * L],
                             start=True, stop=True, skip_group_check=True)

            hs = []
            for d in range(2):
                h_d = hpool.tile([128, S], bf16, tag=f"h{d}", name=f"h{d}")
                hs.append(h_d)
            nc.vector.memset(hs[0][:], 0.0)
            nc.vector.memset(hs[1][:], 0.0)

            # Per step and direction one psum tile [128, 4, S] (2 banks:
            # gates i,f in bank A, g,o in bank B). dir 0 gets bufs=2 so its
            # step-t+1 one-hot matmuls never wait on the step-t gates-ACT;
            # dir 1 (bufs=1, PSUM is full) emits its xg at the end of its
            # block where the WAR wait on its own gates-ACT is already met.
            # The two directions are emitted SERIALLY (f-block then r-block)
            # so they settle a half-period apart: each direction's
            # DVE/ACT work fills the other's h->gates reload latency.
            gbufs = [2, 1]

            def emit_xg(d, t):
                g_ps = gpool.tile([128, 4, S], f32, tag=f"g{d}",
                                  name=f"g{d}_{t}", bufs=gbufs[d])
                off = t if d == 0 else (L + 2 * W - 1 - t)
                q0 = off * S
                for half in range(2):
                    k = half * 4 + q0 // ohtw
                    if k not in primed:
                        # deferred DMA prime; its junk write is wiped by the
                        # bank's first start=True matmul right below
                        primed.add(k)
                        nc.tensor.matmul(g_ps[0:1, 0, 0:1], oht[k][0:1, 0:1],
                                         oht[k][0:1, 0:1], start=True,
                                         stop=True, skip_group_check=True)
                for g in range(4):
                    for half in range(2):
                        lhs = pg[:, ((d * 4 + g) * 2 + half) * 128:
                                    ((d * 4 + g) * 2 + half + 1) * 128]
                        rhs = oht[half * 4 + q0 // ohtw][
                            :, q0 % ohtw:q0 % ohtw + S]
                        # start=True on the first write of each psum bank
                        nc.tensor.matmul(g_ps[:, g, :], lhs, rhs,
                                         start=(half == 0 and g % 2 == 0),
                                         stop=False,
                                         skip_group_check=True)
                return g_ps

            cur = [emit_xg(0, 0), emit_xg(1, 0)]
            for t in range(steps):
                for d in range(2):
                    # --- dir-d block: recurrent MM -> gates ACT -> cell ---
                    for g in range(4):
                        nc.tensor.matmul(
                            cur[d][:, g, :],
                            whh[:, (d * 4 + g) * 128:(d * 4 + g + 1) * 128],
                            hs[d][:], start=False, stop=True,
                            skip_group_check=True)
                    if d == 0 and t + 1 < steps:
                        nxt0 = emit_xg(0, t + 1)
                    acts = apool.tile([128, 4, S], bf16, tag=f"acts{d}",
                                      name=f"acts{d}_{t}")
                    nc.scalar.activation(acts[:], cur[d][:], TANH,
                                         bias=bias0, scale=0.5)
                    yi = acts[:, 0, :]
                    yf = acts[:, 1, :]
                    yg = acts[:, 2, :]
                    yo = acts[:, 3, :]
                    c = cst[d]
                    # c' = (yf+1)*c'*0.5 + (yi+1)*yg
                    A = tpool.tile([128, S], bf16, tag=f"A{d}", name=f"A{d}_{t}")
                    Bt = tpool.tile([128, S], f32, tag=f"B{d}", name=f"B{d}_{t}")
                    nc.vector.scalar_tensor_tensor(A[:], yi, 1.0, yg,
                                                   op0=ADD, op1=MULT)
                    nc.vector.scalar_tensor_tensor(Bt[:], yf, 1.0, c[:],
                                                   op0=ADD, op1=MULT)
                    nc.vector.scalar_tensor_tensor(c[:], Bt[:], 0.5, A[:],
                                                   op0=MULT, op1=ADD)
                    # tanh(c) = tanh(0.5 * c')
                    nc.scalar.activation(tch[d][:], c[:], TANH,
                                         bias=bias0, scale=0.5)
                    h_d = hpool.tile([128, S], bf16, tag=f"h{d}",
                                     name=f"h{d}_{t}")
                    # h' = (yo+1)*tanh(c)
                    nc.vector.scalar_tensor_tensor(h_d[:], yo, 1.0,
                                                   tch[d][:],
                                                   op0=ADD, op1=MULT)
                    hs[d] = h_d
                    # scores: s[:, half, p] += h_half.T @ w_out_dir
                    if t >= W:
                        p = (t - W) if d == 0 else (L + W - 1 - t)
                        for half in range(2):
                            nc.tensor.matmul(
                                scores[:, half, p:p + 1],
                                hs[d][:, half * 128:(half + 1) * 128],
                                wscb[:, d:d + 1], start=False,
                                stop=True, skip_group_check=True)
                    if d == 1 and t + 1 < steps:
                        cur = [nxt0, emit_xg(1, t + 1)]

            # --- epilogue: sigmoid(scores + b_out) and store ---
            nc.scalar.activation(out_sb[:], scores[:], SIGM,
                                 bias=wsc32[:, 0:1])
            nc.sync.dma_start(out_d[:], out_sb[:])

    nc.compile()
    return nc


def _host_prep(inputs, S, L, W):
    """Build per-core in_maps."""
    import ml_dtypes

    bf16 = ml_dtypes.bfloat16

    tokens = np.asarray(inputs["tokens"]).astype(np.int64)
    emb = np.asarray(inputs["embedding"], dtype=np.float32)
    T = tokens.shape[0]
    n2 = (L + 2 * W) * S

    pg_blob = np.zeros((128, 16 * 128), np.float32)
    whh_blob = np.zeros((128, 8 * 128), np.float32)
    for d, sfx in enumerate(("f", "r")):
        w_ih = np.asarray(inputs[f"w_ih_{sfx}"], dtype=np.float32)
        w_hh = np.asarray(inputs[f"w_hh_{sfx}"], dtype=np.float32)
        b = (np.asarray(inputs[f"b_ih_{sfx}"], dtype=np.float32)
             + np.asarray(inputs[f"b_hh_{sfx}"], dtype=np.float32))
        PG = w_ih @ emb.T + b[:, None]          # [512, 256]
        PG[2 * H:3 * H] *= 2.0                  # tanh-trick on g-gate
        whh = w_hh * 0.5                        # h' = 2h compensation
        whh[2 * H:3 * H] *= 2.0                 # tanh-trick on g-gate
        for g in range(4):
            for half in range(2):
                tilev = PG[g * 128:(g + 1) * 128, half * 128:(half + 1) * 128].T
                pg_blob[:, ((d * 4 + g) * 2 + half) * 128:
                           ((d * 4 + g) * 2 + half + 1) * 128] = tilev
            whh_blob[:, (d * 4 + g) * 128:(d * 4 + g + 1) * 128] = \
                whh[g * 128:(g + 1) * 128, :].T

    w_out = np.asarray(inputs["w_out"], dtype=np.float32).reshape(-1)
    b_out = float(np.asarray(inputs["b_out"]).reshape(-1)[0])
    wscb = np.stack([w_out[:H] * 0.5, w_out[H:] * 0.5], axis=1)  # [128, 2]
    wsc32 = np.full((128, 1), b_out, np.float32)

    pg16 = pg_blob.astype(bf16)
    whhb = whh_blob.astype(bf16)
    wscbb = wscb.astype(bf16)

    in_maps = []
    idxg, sg = np.meshgrid(np.arange(L + 2 * W), np.arange(S), indexing="ij")
    colg = (idxg * S + sg).reshape(-1)          # step-major column index
    for core in range(N_CORES):
        base = core * S * L
        pos = (base + sg * L + idxg - W).reshape(-1)
        valid = (pos >= 0) & (pos < T)
        cols = colg[valid]
        toks = tokens[pos[valid]]
        ohc = np.zeros((2, 128, n2), np.float32)
        lo = toks < 128
        ohc[0, toks[lo], cols[lo]] = 1.0
        ohc[1, toks[~lo] - 128, cols[~lo]] = 1.0
        oh = np.concatenate([ohc[0], ohc[1]], axis=1).astype(bf16)  # [128, 2n2]
        in_maps.append({
            "oh": oh,
            "pg": pg16,
            "whh": whhb,
            "wscb": wscbb,
            "wsc32": wsc32,
        })
    return in_maps


_CACHE = {}


def kernel(**inputs):
    from concourse.bass_utils import run_bass_kernel_spmd

    key = (S, L, W)
    if key not in _CACHE:
        _CACHE[key] = _build_nc(S, L, W)
    nc = _CACHE[key]
    in_maps = _host_prep(inputs, S, L, W)
    res = run_bass_kernel_spmd(nc, in_maps, list(range(N_CORES)))
    # out tile is [128, 2, L]: (chunk-row, chunk-half, position);
    # chunk s = half*128 + row, token = core_base + s*L + p
    out = np.concatenate(
        [np.asarray(res.results[c]["out"], dtype=np.float32)
         .reshape(128, 2, L).transpose(1, 0, 2).reshape(-1)
         for c in range(N_CORES)])
    return out


def run_traced(inputs):
    """Run once with NTFF tracing for HW timing / perfetto (dev only)."""
    from concourse.bass_utils import run_bass_kernel_spmd

    key = (S, L, W)
    if key not in _CACHE:
        _CACHE[key] = _build_nc(S, L, W)
    nc = _CACHE[key]
    in_maps = _host_prep(inputs, S, L, W)
    return run_bass_kernel_spmd(nc, in_maps, list(range(N_CORES)), trace=True)
